# revision 14
# baseline (speedup 1.0000x reference)
"""Trainium2 Bass kernel for nn_Butterfly_1580547970089.

Butterfly multiply (n=1024, log_n=10, nstacks=nblocks=1) + bias over a
16384-row batch, data-parallel across 8 NeuronCores (2048 rows each).

Decomposition (per core, features on partitions, batch on the free dim):
  * Stages 0-7 are composed on the host into dense 128x128 bf16 matrices.
    For out-tiles 0-3 stage 8 is ALSO folded into the weights (4 matmuls
    from sources {t, other7(t), P8(t), other7(P8(t))}); out-tiles 4-7 use
    2 matmuls and do stage 8 on DVE. This splits the load: PE ~20 us,
    DVE ~23 us, ACT ~23 us, DMA bus ~26 us (the floor).
  * The per-feature stage-9 bake g_t and tile-diagonal stage-8 coefficient
    are folded into the matmul weights, so vector stages use only fast-mode
    ops (scalar_tensor_tensor has NO DVE fast mode; tensor_scalar 4x and
    tensor_tensor 2x do):
      evac:    u_t = bf16(PSUM_t)                      (ACT, the fp32 pass)
      stage 8: v8_t = u_{P8(t)} * s8_t                 (ts 1-scalar, 4x)
               z_t  = u_t + v8_t                       (tt, 2x)   tiles 4-7
      stage 9: v9_t = y_{P9(t)} * r9_t + b_t           (ts2 on DVE 4x /
               out_t = y_t + v9_t                       ACT activation; tt 2x)
    where y_t = u_t (folded tiles) or z_t (vector tiles). Scalars are
    per-partition [128,1] fp32 APs; ratios are multiplicative (safe), bias
    adds at its natural place.
  * Single bf16 plane in + bf16 out halves DMA bytes (rel err ~5e-3 vs the
    2e-2 gate).
  * Input streams in [128,1024] halves so compute starts ~1.5 us in; output
    DMAs go through gpsimd (SWDGE) to keep the serialized HWDGE overhead
    (~630 ns/DMA) off the input path.

Device per core:
  in:  xT  [1024, 2048] bf16 — row g*128+p = feature, col = batch in shard.
  At   [128, 24*128] bf16: tiles 0-3: 4 lhsT blocks each; tiles 4-7: 2 each.
  coef [128, 24] f32: 0-7 r9, 8-15 bias, 16-19 s8 (tiles 4-7).
  out: outT [1024, 2048] bf16, same layout as xT.
"""
import numpy as np
import ml_dtypes

import concourse.mybir as mybir
import concourse.tile as tile
from concourse import bacc, bass_utils

F32 = mybir.dt.float32
BF16 = mybir.dt.bfloat16
MULT = mybir.AluOpType.mult
ADD = mybir.AluOpType.add

N_CORES = 8
BATCH = 16384
N = 1024
B_CORE = BATCH // N_CORES   # 2048
CHUNK = B_CORE              # single chunk
N_CHUNKS = 1
SUB = 512
N_SUBS = CHUNK // SUB       # 4

S7_PAIRS = [(0, 1), (2, 3), (4, 5), (6, 7)]
S8_PAIRS = [(0, 2), (1, 3), (4, 6), (5, 7)]
S9_PAIRS = [(0, 4), (1, 5), (2, 6), (3, 7)]
OTHER7 = {0: 1, 1: 0, 2: 3, 3: 2, 4: 5, 5: 4, 6: 7, 7: 6}
P8 = {0: 2, 2: 0, 1: 3, 3: 1, 4: 6, 6: 4, 5: 7, 7: 5}
P9 = {0: 4, 4: 0, 1: 5, 5: 1, 2: 6, 6: 2, 3: 7, 7: 3}
FOLDED = (0, 1, 2, 3, 4, 6)  # stage 8 folded into PE (4 matmuls)
VECT = (5, 7)                # stage 8 on DVE (2 matmuls)
SRC = {t: (t, OTHER7[t], P8[t], OTHER7[P8[t]]) for t in FOLDED}

_compiled = {}

_WOFF = {}
for _i, _t in enumerate(FOLDED):
    for _k in range(4):
        _WOFF[(_t, _k)] = (_i * 4 + _k) * 128
for _i, _t in enumerate(VECT):
    for _k in range(2):
        _WOFF[(_t, _k)] = (len(FOLDED) * 4 + _i * 2 + _k) * 128
N_WBLK = len(FOLDED) * 4 + len(VECT) * 2  # 28


def _woff(t, k):
    return _WOFF[(t, k)]


def _emit_kernel(loop_reps=None):
    nc = bacc.Bacc("TRN2", target_bir_lowering=False, debug=False)
    xT = nc.dram_tensor("xT", [N, CHUNK], BF16, kind="ExternalInput").ap()
    At = nc.dram_tensor("At", [128, N_WBLK * 128], BF16, kind="ExternalInput").ap()
    coef = nc.dram_tensor("coef", [128, 24], F32, kind="ExternalInput").ap()
    outT = nc.dram_tensor("outT", [N, CHUNK], BF16, kind="ExternalOutput").ap()

    with tile.TileContext(nc) as tc:
        with (
            tc.tile_pool(name="const", bufs=1) as cpool,
            tc.tile_pool(name="xin", bufs=8) as xpool,
            tc.tile_pool(name="uo", bufs=16) as upool,
            tc.tile_pool(name="v8o", bufs=8) as v8pool,
            tc.tile_pool(name="zo", bufs=8) as zpool,
            tc.tile_pool(name="v9o", bufs=16) as v9pool,
            tc.tile_pool(name="outb", bufs=8) as opool,
            tc.tile_pool(name="ps", bufs=8, space="PSUM") as ppool,
        ):
            at = cpool.tile([128, N_WBLK * 128], BF16, tag="at")
            nc.sync.dma_start(at[:], At[:])
            cf = cpool.tile([128, 24], F32, tag="cf")
            nc.sync.dma_start(cf[:], coef[:])

            def c(col):
                return cf[:, col:col + 1]

            def w(t, k):
                off = _woff(t, k)
                return at[:, off:off + 128]

            ORD = (4, 5, 6, 7, 0, 1, 2, 3)  # VECT tiles first everywhere

            def body():
                xt = [None] * 8
                for g in range(8):
                    xt[g] = xpool.tile([128, CHUNK], BF16, tag="xt",
                                       name=f"xt{g}")
                # stream input: sub-0 slices of all tiles first (compute
                # starts ~3 us in), then the remainder per tile
                for g in ORD:
                    nc.sync.dma_start(xt[g][:, 0:SUB],
                                      xT[g * 128:(g + 1) * 128, 0:SUB])
                for g in ORD:
                    nc.sync.dma_start(xt[g][:, SUB:CHUNK],
                                      xT[g * 128:(g + 1) * 128, SUB:CHUNK])
                ot = [None] * 8
                for g in range(8):
                    ot[g] = opool.tile([128, CHUNK], BF16, tag="ot",
                                       name=f"ot{g}")
                EORD = (5, 7, 4, 6, 0, 1, 2, 3)  # evac & mm order aligned
                for sub in range(N_SUBS):
                    sl = slice(sub * SUB, (sub + 1) * SUB)
                    ps = [None] * 8
                    for t in EORD:
                        ps[t] = ppool.tile([128, SUB], F32, tag="ps",
                                           name=f"ps{t}")
                        if t in VECT:
                            nc.tensor.matmul(ps[t][:], w(t, 0), xt[t][:, sl],
                                             start=True, stop=False)
                            nc.tensor.matmul(ps[t][:], w(t, 1),
                                             xt[OTHER7[t]][:, sl],
                                             start=False, stop=True)
                        else:
                            for k in range(4):
                                nc.tensor.matmul(ps[t][:], w(t, k),
                                                 xt[SRC[t][k]][:, sl],
                                                 start=(k == 0), stop=(k == 3))
                    u = [None] * 8
                    for t in EORD:
                        u[t] = upool.tile([128, SUB], BF16, tag="u",
                                          name=f"u{t}")
                        nc.scalar.copy(u[t][:], ps[t][:])
                    y = [u[t][:] for t in range(8)]
                    v8 = {}
                    for t in VECT:
                        v8[t] = v8pool.tile([128, SUB], BF16, tag="v8",
                                            name=f"v8{t}")
                        nc.vector.tensor_scalar(v8[t][:], u[P8[t]][:],
                                                c(16 + VECT.index(t)), None,
                                                op0=MULT)
                    for t in VECT:
                        z = zpool.tile([128, SUB], BF16, tag="z",
                                       name=f"z{t}")
                        nc.vector.tensor_tensor(z[:], u[t][:], v8[t][:],
                                                op=ADD)
                        y[t] = z[:]
                    # stage 9: prescale+bias then combine
                    v9 = [None] * 8
                    for t in (5, 7, 0, 2, 4, 6, 1, 3):  # v9 of 1,3 need z
                        v9[t] = v9pool.tile([128, SUB], BF16, tag="v9",
                                            name=f"v9{t}")
                        nc.vector.tensor_scalar(v9[t][:], y[P9[t]],
                                                c(t), c(8 + t),
                                                op0=MULT, op1=ADD)
                    for t in ORD:
                        nc.vector.tensor_tensor(ot[t][:, sl], y[t],
                                                v9[t][:], op=ADD)
                    # output DMA: [0:1024] after sub 1, then per-sub slices
                    # (finer at the end to shorten the tail)
                    if sub >= 1:
                        lo = 0 if sub == 1 else sub * SUB
                        osl = slice(lo, (sub + 1) * SUB)
                        for t in ORD:
                            nc.sync.dma_start(
                                outT[t * 128:(t + 1) * 128, osl],
                                ot[t][:, osl])

            if loop_reps is not None:
                with tc.For_i(0, loop_reps, 1,
                              hint_engines=(mybir.EngineType.PE,
                                            mybir.EngineType.DVE,
                                            mybir.EngineType.Activation)):
                    body()
            else:
                body()

    nc.compile()
    return nc


def _get_compiled(loop_reps=None):
    if loop_reps not in _compiled:
        _compiled[loop_reps] = _emit_kernel(loop_reps)
    return _compiled[loop_reps]


def _build_A(twiddle):
    A = np.zeros((8, 128, 128), np.float64)
    for h in range(8):
        M = np.eye(128, dtype=np.float64)
        for idx in range(7):
            s = 1 << idx
            tw = twiddle[0, 0, idx].astype(np.float64).reshape(512 // s, s, 2, 2)
            tw_h = tw[h * (64 // s):(h + 1) * (64 // s)]
            Mv = M.reshape(64 // s, 2, s, 128)
            top, bot = Mv[:, 0], Mv[:, 1]
            M = np.stack(
                [tw_h[:, :, 0, 0][..., None] * top + tw_h[:, :, 0, 1][..., None] * bot,
                 tw_h[:, :, 1, 0][..., None] * top + tw_h[:, :, 1, 1][..., None] * bot],
                axis=1).reshape(128, 128)
        A[h] = M
    return A


def _coef_parts(twiddle):
    t8 = twiddle[0, 0, 8].reshape(2, 256, 2, 2).astype(np.float64)
    t9 = twiddle[0, 0, 9].reshape(512, 2, 2).astype(np.float64)
    c8d = np.zeros((8, 128)); c8o = np.zeros((8, 128))
    for gi, (p_, q_) in enumerate(S8_PAIRS):
        G, hp = divmod(gi, 2)
        cc = t8[G, hp * 128:(hp + 1) * 128]
        c8d[p_], c8o[p_] = cc[:, 0, 0], cc[:, 0, 1]
        c8d[q_], c8o[q_] = cc[:, 1, 1], cc[:, 1, 0]
    g = np.zeros((8, 128)); r9 = np.zeros((8, 128))
    for a, b in S9_PAIRS:
        e = t9[a * 128:(a + 1) * 128]
        g[a], g[b] = e[:, 0, 0], e[:, 1, 1]
        r9[a] = e[:, 0, 1] / e[:, 1, 1]
        r9[b] = e[:, 1, 0] / e[:, 0, 0]
    return c8d, c8o, g, r9


def _build_weights(twiddle):
    """At [128, 24*128] bf16 per _woff layout (lhsT blocks)."""
    A = _build_A(twiddle)
    t7 = twiddle[0, 0, 7].reshape(4, 128, 2, 2).astype(np.float64)
    B = np.zeros((8, 128, 128)); C = np.zeros((8, 128, 128))
    for gi, (p, q) in enumerate(S7_PAIRS):
        B[p] = np.diag(t7[gi, :, 0, 0]) @ A[p]
        C[p] = np.diag(t7[gi, :, 0, 1]) @ A[q]
        B[q] = np.diag(t7[gi, :, 1, 1]) @ A[q]
        C[q] = np.diag(t7[gi, :, 1, 0]) @ A[p]
    c8d, c8o, g, _ = _coef_parts(twiddle)
    At = np.zeros((128, N_WBLK * 128), ml_dtypes.bfloat16)
    for t in FOLDED:
        pt = P8[t]
        Ws = [
            (g[t] * c8d[t])[:, None] * B[t],
            (g[t] * c8d[t])[:, None] * C[t],
            (g[t] * c8o[t])[:, None] * B[pt],
            (g[t] * c8o[t])[:, None] * C[pt],
        ]
        for k, W in enumerate(Ws):
            off = _woff(t, k)
            At[:, off:off + 128] = W.T.astype(ml_dtypes.bfloat16)
    for t in VECT:
        alpha = g[t] * c8d[t]
        for k, W in enumerate((alpha[:, None] * B[t], alpha[:, None] * C[t])):
            off = _woff(t, k)
            At[:, off:off + 128] = W.T.astype(ml_dtypes.bfloat16)
    return At


def _build_coef(twiddle, bias):
    c8d, c8o, g, r9 = _coef_parts(twiddle)
    coef = np.zeros((128, 24), np.float32)
    coef[:, 0:8] = r9.T
    coef[:, 8:16] = np.asarray(bias, np.float64).reshape(8, 128).T
    for i, t in enumerate(VECT):
        alpha_p = g[P8[t]] * c8d[P8[t]]
        coef[:, 16 + i] = g[t] * c8o[t] / alpha_p
    return coef


def _build_xT(shard):
    """shard [B_CORE, 1024] fp32 -> [1024, B_CORE] bf16 (transposed)."""
    return np.ascontiguousarray(shard.T).astype(ml_dtypes.bfloat16)


def kernel(input, twiddle, bias):
    input = np.asarray(input)
    twiddle = np.asarray(twiddle)
    bias = np.asarray(bias)
    nc = _get_compiled()

    At = _build_weights(twiddle)
    coef = _build_coef(twiddle, bias)
    in_maps = []
    for cid in range(N_CORES):
        shard = input[cid * B_CORE:(cid + 1) * B_CORE, :]
        in_maps.append({"xT": _build_xT(shard), "At": At, "coef": coef})

    res = bass_utils.run_bass_kernel_spmd(nc, in_maps,
                                          core_ids=list(range(N_CORES)))
    out = np.empty((BATCH, N), np.float32)
    for cid in range(N_CORES):
        o = res.results[cid]["outT"]  # [1024, B_CORE] bf16
        out[cid * B_CORE:(cid + 1) * B_CORE, :] = o.T.astype(np.float32)
    return out


# revision 16
# speedup vs baseline: 1.0796x; 1.0796x over previous
"""Trainium2 Bass kernel for nn_Butterfly_1580547970089.

Butterfly multiply (n=1024, log_n=10, nstacks=nblocks=1) + bias over a
16384-row batch, data-parallel across 8 NeuronCores (2048 rows each).

Decomposition (per core, features on partitions, batch on the free dim):
  * Stages 0-7 are composed on the host into dense 128x128 bf16 matrices.
    For out-tiles 0-3 stage 8 is ALSO folded into the weights (4 matmuls
    from sources {t, other7(t), P8(t), other7(P8(t))}); out-tiles 4-7 use
    2 matmuls and do stage 8 on DVE. This splits the load: PE ~20 us,
    DVE ~23 us, ACT ~23 us, DMA bus ~26 us (the floor).
  * The per-feature stage-9 bake g_t and tile-diagonal stage-8 coefficient
    are folded into the matmul weights, so vector stages use only fast-mode
    ops (scalar_tensor_tensor has NO DVE fast mode; tensor_scalar 4x and
    tensor_tensor 2x do):
      evac:    u_t = bf16(PSUM_t)                      (ACT, the fp32 pass)
      stage 8: v8_t = u_{P8(t)} * s8_t                 (ts 1-scalar, 4x)
               z_t  = u_t + v8_t                       (tt, 2x)   tiles 4-7
      stage 9: v9_t = y_{P9(t)} * r9_t + b_t           (ts2 on DVE 4x /
               out_t = y_t + v9_t                       ACT activation; tt 2x)
    where y_t = u_t (folded tiles) or z_t (vector tiles). Scalars are
    per-partition [128,1] fp32 APs; ratios are multiplicative (safe), bias
    adds at its natural place.
  * Single bf16 plane in + bf16 out halves DMA bytes (rel err ~5e-3 vs the
    2e-2 gate).
  * Input streams in [128,1024] halves so compute starts ~1.5 us in; output
    DMAs go through gpsimd (SWDGE) to keep the serialized HWDGE overhead
    (~630 ns/DMA) off the input path.

Device per core:
  in:  xT  [1024, 2048] bf16 — row g*128+p = feature, col = batch in shard.
  At   [128, 24*128] bf16: tiles 0-3: 4 lhsT blocks each; tiles 4-7: 2 each.
  coef [128, 24] f32: 0-7 r9, 8-15 bias, 16-19 s8 (tiles 4-7).
  out: outT [1024, 2048] bf16, same layout as xT.
"""
import numpy as np
import ml_dtypes

import concourse.mybir as mybir
import concourse.tile as tile
from concourse import bacc, bass_utils

F32 = mybir.dt.float32
BF16 = mybir.dt.bfloat16
MULT = mybir.AluOpType.mult
ADD = mybir.AluOpType.add

N_CORES = 8
BATCH = 16384
N = 1024
B_CORE = BATCH // N_CORES   # 2048
CHUNK = B_CORE              # single chunk
N_CHUNKS = 1
SUB = 512
N_SUBS = CHUNK // SUB       # 4

S7_PAIRS = [(0, 1), (2, 3), (4, 5), (6, 7)]
S8_PAIRS = [(0, 2), (1, 3), (4, 6), (5, 7)]
S9_PAIRS = [(0, 4), (1, 5), (2, 6), (3, 7)]
OTHER7 = {0: 1, 1: 0, 2: 3, 3: 2, 4: 5, 5: 4, 6: 7, 7: 6}
P8 = {0: 2, 2: 0, 1: 3, 3: 1, 4: 6, 6: 4, 5: 7, 7: 5}
P9 = {0: 4, 4: 0, 1: 5, 5: 1, 2: 6, 6: 2, 3: 7, 7: 3}
FOLDED = (0, 1, 2, 3, 4, 6)  # stage 8 folded into PE (4 matmuls)
VECT = (5, 7)                # stage 8 on DVE (2 matmuls)
SRC = {t: (t, OTHER7[t], P8[t], OTHER7[P8[t]]) for t in FOLDED}

_compiled = {}

_WOFF = {}
for _i, _t in enumerate(FOLDED):
    for _k in range(4):
        _WOFF[(_t, _k)] = (_i * 4 + _k) * 128
for _i, _t in enumerate(VECT):
    for _k in range(2):
        _WOFF[(_t, _k)] = (len(FOLDED) * 4 + _i * 2 + _k) * 128
N_WBLK = len(FOLDED) * 4 + len(VECT) * 2  # 28


def _woff(t, k):
    return _WOFF[(t, k)]


def _emit_kernel(loop_reps=None):
    nc = bacc.Bacc("TRN2", target_bir_lowering=False, debug=False)
    xT = nc.dram_tensor("xT", [N, CHUNK], BF16, kind="ExternalInput").ap()
    At = nc.dram_tensor("At", [128, N_WBLK * 128], BF16, kind="ExternalInput").ap()
    coef = nc.dram_tensor("coef", [128, 24], F32, kind="ExternalInput").ap()
    outT = nc.dram_tensor("outT", [N, CHUNK], BF16, kind="ExternalOutput").ap()

    with tile.TileContext(nc) as tc:
        with (
            tc.tile_pool(name="const", bufs=1) as cpool,
            tc.tile_pool(name="xin", bufs=8) as xpool,
            tc.tile_pool(name="uo", bufs=16) as upool,
            tc.tile_pool(name="v8o", bufs=8) as v8pool,
            tc.tile_pool(name="zo", bufs=8) as zpool,
            tc.tile_pool(name="v9o", bufs=16) as v9pool,
            tc.tile_pool(name="outb", bufs=8) as opool,
            tc.tile_pool(name="ps", bufs=8, space="PSUM") as ppool,
        ):
            at = cpool.tile([128, N_WBLK * 128], BF16, tag="at")
            nc.sync.dma_start(at[:], At[:])
            cf = cpool.tile([128, 24], F32, tag="cf")
            nc.sync.dma_start(cf[:], coef[:])

            def c(col):
                return cf[:, col:col + 1]

            def w(t, k):
                off = _woff(t, k)
                return at[:, off:off + 128]

            ORD = (4, 5, 6, 7, 0, 1, 2, 3)  # VECT tiles first everywhere

            def body():
                xt = [None] * 8
                for g in range(8):
                    xt[g] = xpool.tile([128, CHUNK], BF16, tag="xt",
                                       name=f"xt{g}")
                # stream input: sub-0 slices of all tiles first (compute
                # starts ~3 us in), then the remainder per tile
                for g in ORD:
                    nc.sync.dma_start(xt[g][:, 0:SUB],
                                      xT[g * 128:(g + 1) * 128, 0:SUB])
                for g in ORD:
                    nc.sync.dma_start(xt[g][:, SUB:CHUNK],
                                      xT[g * 128:(g + 1) * 128, SUB:CHUNK])
                ot = [None] * 8
                for g in range(8):
                    ot[g] = opool.tile([128, CHUNK], BF16, tag="ot",
                                       name=f"ot{g}")
                # evac & mm order: VECT (5,7) first to unlock stage 8, then
                # S9 partners of 5,7 (1,3) so out-pairs (1,5),(3,7) complete
                # early, then pairs (0,4),(2,6) — outputs drain pairwise
                EORD = (5, 7, 1, 3, 0, 4, 2, 6)
                for sub in range(N_SUBS):
                    sl = slice(sub * SUB, (sub + 1) * SUB)
                    ps = [None] * 8
                    for t in EORD:
                        ps[t] = ppool.tile([128, SUB], F32, tag="ps",
                                           name=f"ps{t}")
                        if t in VECT:
                            nc.tensor.matmul(ps[t][:], w(t, 0), xt[t][:, sl],
                                             start=True, stop=False)
                            nc.tensor.matmul(ps[t][:], w(t, 1),
                                             xt[OTHER7[t]][:, sl],
                                             start=False, stop=True)
                        else:
                            for k in range(4):
                                nc.tensor.matmul(ps[t][:], w(t, k),
                                                 xt[SRC[t][k]][:, sl],
                                                 start=(k == 0), stop=(k == 3))
                    u = [None] * 8
                    for t in EORD:
                        u[t] = upool.tile([128, SUB], BF16, tag="u",
                                          name=f"u{t}")
                        nc.scalar.copy(u[t][:], ps[t][:])
                    y = [u[t][:] for t in range(8)]
                    v8 = {}
                    for t in VECT:
                        v8[t] = v8pool.tile([128, SUB], BF16, tag="v8",
                                            name=f"v8{t}")
                        nc.vector.tensor_scalar(v8[t][:], u[P8[t]][:],
                                                c(16 + VECT.index(t)), None,
                                                op0=MULT)
                    for t in VECT:
                        z = zpool.tile([128, SUB], BF16, tag="z",
                                       name=f"z{t}")
                        nc.vector.tensor_tensor(z[:], u[t][:], v8[t][:],
                                                op=ADD)
                        y[t] = z[:]
                    # stage 9: prescale+bias then combine, in pair order so
                    # each out-pair fires as soon as its two y's exist
                    PORD = (5, 1, 7, 3, 0, 4, 2, 6)
                    v9 = [None] * 8
                    for t in PORD:
                        v9[t] = v9pool.tile([128, SUB], BF16, tag="v9",
                                            name=f"v9{t}")
                        nc.vector.tensor_scalar(v9[t][:], y[P9[t]],
                                                c(t), c(8 + t),
                                                op0=MULT, op1=ADD)
                    for t in PORD:
                        nc.vector.tensor_tensor(ot[t][:, sl], y[t],
                                                v9[t][:], op=ADD)
                    # output DMA: [0:1024] after sub 1, then per-sub slices
                    # (finer at the end to shorten the tail)
                    if sub >= 1:
                        lo = 0 if sub == 1 else sub * SUB
                        osl = slice(lo, (sub + 1) * SUB)
                        for t in PORD:
                            nc.sync.dma_start(
                                outT[t * 128:(t + 1) * 128, osl],
                                ot[t][:, osl])

            if loop_reps is not None:
                with tc.For_i(0, loop_reps, 1,
                              hint_engines=(mybir.EngineType.PE,
                                            mybir.EngineType.DVE,
                                            mybir.EngineType.Activation)):
                    body()
            else:
                body()

    nc.compile()
    return nc


def _get_compiled(loop_reps=None):
    if loop_reps not in _compiled:
        _compiled[loop_reps] = _emit_kernel(loop_reps)
    return _compiled[loop_reps]


def _build_A(twiddle):
    A = np.zeros((8, 128, 128), np.float64)
    for h in range(8):
        M = np.eye(128, dtype=np.float64)
        for idx in range(7):
            s = 1 << idx
            tw = twiddle[0, 0, idx].astype(np.float64).reshape(512 // s, s, 2, 2)
            tw_h = tw[h * (64 // s):(h + 1) * (64 // s)]
            Mv = M.reshape(64 // s, 2, s, 128)
            top, bot = Mv[:, 0], Mv[:, 1]
            M = np.stack(
                [tw_h[:, :, 0, 0][..., None] * top + tw_h[:, :, 0, 1][..., None] * bot,
                 tw_h[:, :, 1, 0][..., None] * top + tw_h[:, :, 1, 1][..., None] * bot],
                axis=1).reshape(128, 128)
        A[h] = M
    return A


def _coef_parts(twiddle):
    t8 = twiddle[0, 0, 8].reshape(2, 256, 2, 2).astype(np.float64)
    t9 = twiddle[0, 0, 9].reshape(512, 2, 2).astype(np.float64)
    c8d = np.zeros((8, 128)); c8o = np.zeros((8, 128))
    for gi, (p_, q_) in enumerate(S8_PAIRS):
        G, hp = divmod(gi, 2)
        cc = t8[G, hp * 128:(hp + 1) * 128]
        c8d[p_], c8o[p_] = cc[:, 0, 0], cc[:, 0, 1]
        c8d[q_], c8o[q_] = cc[:, 1, 1], cc[:, 1, 0]
    g = np.zeros((8, 128)); r9 = np.zeros((8, 128))
    for a, b in S9_PAIRS:
        e = t9[a * 128:(a + 1) * 128]
        g[a], g[b] = e[:, 0, 0], e[:, 1, 1]
        r9[a] = e[:, 0, 1] / e[:, 1, 1]
        r9[b] = e[:, 1, 0] / e[:, 0, 0]
    return c8d, c8o, g, r9


def _build_weights(twiddle):
    """At [128, 24*128] bf16 per _woff layout (lhsT blocks)."""
    A = _build_A(twiddle)
    t7 = twiddle[0, 0, 7].reshape(4, 128, 2, 2).astype(np.float64)
    B = np.zeros((8, 128, 128)); C = np.zeros((8, 128, 128))
    for gi, (p, q) in enumerate(S7_PAIRS):
        B[p] = np.diag(t7[gi, :, 0, 0]) @ A[p]
        C[p] = np.diag(t7[gi, :, 0, 1]) @ A[q]
        B[q] = np.diag(t7[gi, :, 1, 1]) @ A[q]
        C[q] = np.diag(t7[gi, :, 1, 0]) @ A[p]
    c8d, c8o, g, _ = _coef_parts(twiddle)
    At = np.zeros((128, N_WBLK * 128), ml_dtypes.bfloat16)
    for t in FOLDED:
        pt = P8[t]
        Ws = [
            (g[t] * c8d[t])[:, None] * B[t],
            (g[t] * c8d[t])[:, None] * C[t],
            (g[t] * c8o[t])[:, None] * B[pt],
            (g[t] * c8o[t])[:, None] * C[pt],
        ]
        for k, W in enumerate(Ws):
            off = _woff(t, k)
            At[:, off:off + 128] = W.T.astype(ml_dtypes.bfloat16)
    for t in VECT:
        alpha = g[t] * c8d[t]
        for k, W in enumerate((alpha[:, None] * B[t], alpha[:, None] * C[t])):
            off = _woff(t, k)
            At[:, off:off + 128] = W.T.astype(ml_dtypes.bfloat16)
    return At


def _build_coef(twiddle, bias):
    c8d, c8o, g, r9 = _coef_parts(twiddle)
    coef = np.zeros((128, 24), np.float32)
    coef[:, 0:8] = r9.T
    coef[:, 8:16] = np.asarray(bias, np.float64).reshape(8, 128).T
    for i, t in enumerate(VECT):
        alpha_p = g[P8[t]] * c8d[P8[t]]
        coef[:, 16 + i] = g[t] * c8o[t] / alpha_p
    return coef


def _build_xT(shard):
    """shard [B_CORE, 1024] fp32 -> [1024, B_CORE] bf16 (transposed)."""
    return np.ascontiguousarray(shard.T).astype(ml_dtypes.bfloat16)


def kernel(input, twiddle, bias):
    input = np.asarray(input)
    twiddle = np.asarray(twiddle)
    bias = np.asarray(bias)
    nc = _get_compiled()

    At = _build_weights(twiddle)
    coef = _build_coef(twiddle, bias)
    in_maps = []
    for cid in range(N_CORES):
        shard = input[cid * B_CORE:(cid + 1) * B_CORE, :]
        in_maps.append({"xT": _build_xT(shard), "At": At, "coef": coef})

    res = bass_utils.run_bass_kernel_spmd(nc, in_maps,
                                          core_ids=list(range(N_CORES)))
    out = np.empty((BATCH, N), np.float32)
    for cid in range(N_CORES):
        o = res.results[cid]["outT"]  # [1024, B_CORE] bf16
        out[cid * B_CORE:(cid + 1) * B_CORE, :] = o.T.astype(np.float32)
    return out
